# revision 9
# baseline (speedup 1.0000x reference)
"""GQA attention layer with RoPE + KV cache (LLaMA-70B style) on 8 Trainium2 cores.

Sharding: tensor-parallel over heads. Each core owns 4 q heads + 1 kv head
(one whole GQA group) and the matching column-shards of wq/wk/wv and row-shard
of wo. Every core sees the full batch/sequence, computes its partial output
projection, and the host sums the 8 partials (the TP all-reduce) and reshapes.

Device pipeline per core (all matmuls bf16, fp32 psum accumulation):
  1. Projections: q^T/k^T/v^T = W^T x^T streamed over token blocks; RoPE
     applied on DVE with a host-side head-dim permutation that makes the
     rotate-half partner reachable by stream_shuffle (+/-16 within each
     32-partition quadrant); v transposed to natural [tok, D] layout on PE.
  2. Attention per (batch, head): s = q^T.T @ k^T in [qt,kt] layout with the
     causal staircase masked by accumulating a -30000 bias tile via an
     identity matmul; exp+row-sum fused on ScalarE (accum_out); probabilities
     normalized on DVE, transposed on PE, then av^T = v.T @ p^T accumulated
     over kt blocks.
  3. Output projection: out[tok, E] += av_h^T.T @ wo_h accumulated over the
     4 local heads; partial written to HBM in fp32.
"""
import numpy as np
import ml_dtypes

B, S, E = 2, 2048, 4096
HQ, HKV, D = 32, 8, 128
NCORE = 8
HL = HQ // NCORE          # 4 local q heads
TOK = B * S               # 4096 tokens, b-major
SCALE = 1.0 / np.sqrt(E)
TB = 512                  # token block for projections
NTB = TOK // TB           # 8
NEG = -30000.0

_CACHE = {}


def _perm_src():
    """partition p <- original head-dim index, RoPE pairs at (p, p+16) in quadrant."""
    perm = np.zeros(D, dtype=np.int64)
    for j in range(64):
        b_, s_ = j // 16, j % 16
        perm[32 * b_ + s_] = 2 * j
        perm[32 * b_ + 16 + s_] = 2 * j + 1
    return perm


def _build():
    import concourse.mybir as mybir
    import concourse.tile as tile
    from concourse import bacc
    from concourse.masks import make_identity

    bf = mybir.dt.bfloat16
    f32 = mybir.dt.float32
    Exp = mybir.ActivationFunctionType.Exp
    mult = mybir.AluOpType.mult
    add = mybir.AluOpType.add
    AX = mybir.AxisListType.X

    nc = bacc.Bacc(debug=False)
    with tile.TileContext(nc) as tc:
        with tc.tile_pool(name="dram", bufs=1, space="DRAM") as dram:
            d_xt = dram.tile([NTB, E, TB], bf, kind="ExternalInput", name="xt")
            d_wq = dram.tile([E, HL * D], bf, kind="ExternalInput", name="wq")
            d_wk = dram.tile([E, D], bf, kind="ExternalInput", name="wk")
            d_wv = dram.tile([E, D], bf, kind="ExternalInput", name="wv")
            d_wo = dram.tile([HL * D, E], bf, kind="ExternalInput", name="wo")
            d_cos = dram.tile([D, S], bf, kind="ExternalInput", name="cosg")
            d_sin = dram.tile([D, S], bf, kind="ExternalInput", name="sing")
            d_msk = dram.tile([128, 128], bf, kind="ExternalInput", name="maskb")
            d_out = dram.tile([TOK, E], f32, kind="ExternalOutput", name="out")

            from contextlib import ExitStack
            _stack = ExitStack()
            const = _stack.enter_context(tc.tile_pool(name="const", bufs=1))
            wpool = _stack.enter_context(tc.tile_pool(name="wpool", bufs=1))
            qkv = _stack.enter_context(tc.tile_pool(name="qkv", bufs=1))

            ident = const.tile([128, 128], bf, tag="ident")
            make_identity(nc, ident[:])
            maskb = const.tile([128, 128], bf, tag="maskb")
            nc.sync.dma_start(out=maskb[:], in_=d_msk[:])
            wq_sb = wpool.tile([128, E // 128, HL * D], bf, tag="wq")
            wk_sb = wpool.tile([128, E // 128, D], bf, tag="wk")
            wv_sb = wpool.tile([128, E // 128, D], bf, tag="wv")
            nc.sync.dma_start(out=wq_sb[:], in_=d_wq[:].rearrange("(o p) m -> p o m", p=128))
            nc.sync.dma_start(out=wk_sb[:], in_=d_wk[:].rearrange("(o p) m -> p o m", p=128))
            nc.sync.dma_start(out=wv_sb[:], in_=d_wv[:].rearrange("(o p) m -> p o m", p=128))

            # phase-1 outputs (resident through the whole kernel)
            qT = [qkv.tile([D, TOK], bf, tag=f"qT{h}", name=f"qT{h}") for h in range(HL)]
            kT = qkv.tile([D, TOK], bf, tag="kT")
            vn = qkv.tile([128, TOK], bf, tag="vn")  # [tok%128, 128*tile + d]

            NE = E // 128  # 32 contraction chunks
            shuf = [(i + 16) % 32 for i in range(32)]

            # ---------------- Phase 1: projections + RoPE ----------------
            with (
                tc.tile_pool(name="xt", bufs=NE + 4) as xtp,
                tc.tile_pool(name="trig", bufs=1) as trig,
                tc.tile_pool(name="prj", bufs=2) as prj,
                tc.tile_pool(name="pps", bufs=1, space="PSUM") as pps,
                tc.tile_pool(name="vtp", bufs=1, space="PSUM") as vtp,
            ):
                cosg = trig.tile([D, S], bf, tag="cosg")
                sing = trig.tile([D, S], bf, tag="sing")
                nc.sync.dma_start(out=cosg[:], in_=d_cos[:])
                nc.sync.dma_start(out=sing[:], in_=d_sin[:])
                for t in range(NTB):
                    xts = []
                    for j in range(NE):
                        xt_t = xtp.tile([128, TB], bf, tag="xt")
                        nc.sync.dma_start(out=xt_t[:], in_=d_xt[t, 128 * j:128 * (j + 1), :])
                        xts.append(xt_t)
                    outs = []  # (psum, kind, h)
                    for h in range(HL):
                        ps = pps.tile([128, TB], f32, tag=f"q{h}")
                        for j in range(NE):
                            nc.tensor.matmul(ps[:], wq_sb[:, j, 128 * h:128 * (h + 1)],
                                             xts[j][:], start=(j == 0), stop=(j == NE - 1))
                        outs.append((ps, "q", h))
                    psk = pps.tile([128, TB], f32, tag="k")
                    for j in range(NE):
                        nc.tensor.matmul(psk[:], wk_sb[:, j, :], xts[j][:],
                                         start=(j == 0), stop=(j == NE - 1))
                    outs.append((psk, "k", 0))
                    psv = pps.tile([128, TB], f32, tag="v")
                    for j in range(NE):
                        nc.tensor.matmul(psv[:], wv_sb[:, j, :], xts[j][:],
                                         start=(j == 0), stop=(j == NE - 1))

                    sl = slice(TB * t, TB * (t + 1))
                    psl = slice((TB * t) % S, (TB * t) % S + TB)
                    for ps, kind, h in outs:
                        dst = qT[h] if kind == "q" else kT
                        u = prj.tile([128, TB], bf, tag="u")
                        nc.vector.tensor_tensor(out=u[:], in0=ps[:], in1=cosg[:, psl], op=mult)
                        sh = prj.tile([128, TB], f32, tag="sh")
                        nc.vector.stream_shuffle(out=sh[:], in_=ps[:], mask=shuf)
                        w_ = prj.tile([128, TB], bf, tag="w")
                        nc.vector.tensor_tensor(out=w_[:], in0=sh[:], in1=sing[:, psl], op=mult)
                        nc.vector.tensor_tensor(out=dst[:, sl], in0=u[:], in1=w_[:], op=add)
                    # v: evacuate v^T then transpose to natural [tok, D]
                    vte = prj.tile([128, TB], bf, tag="vte")
                    nc.scalar.copy(out=vte[:], in_=psv[:])
                    for q in range(TB // 128):
                        vt_ps = vtp.tile([128, 128], bf, tag="vt")
                        nc.tensor.transpose(vt_ps[:], vte[:, 128 * q:128 * (q + 1)], ident[:])
                        gt = t * (TB // 128) + q  # global token tile
                        nc.vector.tensor_copy(out=vn[:, 128 * gt:128 * (gt + 1)], in_=vt_ps[:])

            # ---------------- Phase 2 + 3 per batch ----------------
            wo_pool = _stack.enter_context(tc.tile_pool(name="wo_pool", bufs=1))
            wo_sb = wo_pool.tile([128, HL, E], bf, tag="wo")
            nc.sync.dma_start(out=wo_sb[:], in_=d_wo[:].rearrange("(h p) e -> p h e", p=128))
            for b_ in range(B):
                boff = b_ * S
                avT = [qkv.tile([D, S], bf, tag=f"avT{h}", name=f"avT{h}_{b_}") for h in range(HL)]
                with (
                    tc.tile_pool(name="att", bufs=2) as att,
                    tc.tile_pool(name="sps", bufs=2, space="PSUM") as sps,
                    tc.tile_pool(name="tps", bufs=2, space="PSUM") as tps,
                    tc.tile_pool(name="aps", bufs=2, space="PSUM") as aps,
                ):
                    for h in range(HL):
                        for M in range(4):  # 512-token q supertile
                            pch = []
                            for i in range(4):  # 128-token q tile
                                m = 4 * M + i
                                q0 = 512 * M + 128 * i
                                p_sb = att.tile([128, 2048], bf, tag=f"p{i}")
                                dacc = att.tile([128, 4], f32, tag=f"dacc{i}")
                                for c in range(M + 1):
                                    width = 512 if c < M else 128 * (i + 1)
                                    s_ps = sps.tile([128, 512], f32, tag="s")
                                    nc.tensor.matmul(
                                        s_ps[:, :width],
                                        qT[h][:, boff + q0: boff + q0 + 128],
                                        kT[:, boff + 512 * c: boff + 512 * c + width],
                                        start=True, stop=(c < M))
                                    if c == M:
                                        nc.tensor.matmul(
                                            s_ps[:, 128 * i:128 * (i + 1)],
                                            ident[:], maskb[:], start=False, stop=True)
                                    nc.scalar.activation(
                                        out=p_sb[:, 512 * c: 512 * c + width],
                                        in_=s_ps[:, :width], func=Exp, scale=float(SCALE),
                                        accum_out=dacc[:, c:c + 1])
                                if i < 3:
                                    nc.vector.memset(p_sb[:, 512 * M + 128 * (i + 1): 512 * (M + 1)], 0.0)
                                dsum = att.tile([128, 1], f32, tag=f"ds{i}")
                                if M > 0:
                                    nc.vector.tensor_reduce(out=dsum[:], in_=dacc[:, :M + 1], axis=AX, op=add)
                                else:
                                    nc.vector.tensor_copy(out=dsum[:], in_=dacc[:, 0:1])
                                rcp = att.tile([128, 1], f32, tag=f"rcp{i}")
                                nc.vector.reciprocal(rcp[:], dsum[:])
                                nc.vector.tensor_scalar(
                                    out=p_sb[:, :512 * (M + 1)], in0=p_sb[:, :512 * (M + 1)],
                                    scalar1=rcp[:, 0:1], scalar2=None, op0=mult)
                                pch.append(p_sb)
                            av_ps = aps.tile([128, 512], f32, tag="av")
                            nk = 4 * (M + 1)
                            for jkt in range(nk):
                                pt_ps = tps.tile([128, 512], bf, tag="pt")
                                for i in range(4):
                                    nc.tensor.transpose(
                                        pt_ps[:, 128 * i:128 * (i + 1)],
                                        pch[i][:, 128 * jkt:128 * (jkt + 1)], ident[:])
                                pt_sb = att.tile([128, 512], bf, tag="pt_sb")
                                nc.vector.tensor_copy(out=pt_sb[:], in_=pt_ps[:])
                                gkt = (boff // 128) + jkt
                                nc.tensor.matmul(av_ps[:], vn[:, 128 * gkt:128 * (gkt + 1)],
                                                 pt_sb[:], start=(jkt == 0), stop=(jkt == nk - 1))
                            nc.vector.tensor_copy(out=avT[h][:, 512 * M:512 * (M + 1)], in_=av_ps[:])

                # ---- Phase 3: out[tok, E] = sum_h avT_h.T @ wo_h ----
                with (
                    tc.tile_pool(name="wop", bufs=3) as wop,
                    tc.tile_pool(name="ops", bufs=5, space="PSUM") as ops,
                ):
                    for tt in range(S // 128):
                        for e in range(E // 512):
                            o_ps = ops.tile([128, 512], f32, tag="o")
                            for h in range(HL):
                                nc.tensor.matmul(
                                    o_ps[:], avT[h][:, 128 * tt:128 * (tt + 1)],
                                    wo_sb[:, h, 512 * e:512 * (e + 1)],
                                    start=(h == 0), stop=(h == HL - 1))
                            o_sb = wop.tile([128, 512], f32, tag="o_sb")
                            nc.vector.tensor_copy(out=o_sb[:], in_=o_ps[:])
                            nc.sync.dma_start(
                                out=d_out[boff + 128 * tt: boff + 128 * (tt + 1),
                                          512 * e:512 * (e + 1)],
                                in_=o_sb[:])
            _stack.close()
    names = {k: t.tensor.name for k, t in dict(
        xt=d_xt, wq=d_wq, wk=d_wk, wv=d_wv, wo=d_wo,
        cosg=d_cos, sing=d_sin, maskb=d_msk, out=d_out).items()}
    nc.compile()
    return nc, names


def kernel(x, input_pos, freqs_cis, wq, wk, wv, wo, k_cache=None, v_cache=None, **_):
    from concourse.bass_utils import run_bass_kernel_spmd

    x = np.asarray(x, dtype=np.float32)
    freqs_cis = np.asarray(freqs_cis, dtype=np.float32)
    wq = np.asarray(wq, dtype=np.float32)
    wk = np.asarray(wk, dtype=np.float32)
    wv = np.asarray(wv, dtype=np.float32)
    wo = np.asarray(wo, dtype=np.float32)
    bf = ml_dtypes.bfloat16

    if "nc" not in _CACHE:
        _CACHE["nc"] = _build()
    nc, names = _CACHE["nc"]

    perm = _perm_src()
    # xt blocks: [NTB, E, TB] bf16
    x2 = x.reshape(TOK, E)
    xt = np.ascontiguousarray(
        x2.T.reshape(E, NTB, TB).transpose(1, 0, 2)).astype(bf)
    # cos/sin grids in permuted-partition layout
    j_of_p = 16 * (np.arange(D) // 32) + (np.arange(D) % 16)
    sign = np.where((np.arange(D) % 32) < 16, -1.0, 1.0).astype(np.float32)
    cosg = freqs_cis[:, :, 0].T[j_of_p, :].astype(bf)                  # [D, S]
    sing = (freqs_cis[:, :, 1].T[j_of_p, :] * sign[:, None]).astype(bf)
    maskb = np.where(np.arange(128)[None, :] <= np.arange(128)[:, None],
                     0.0, NEG).astype(bf)

    wq4 = wq.reshape(E, HQ, D)
    wk4 = wk.reshape(E, HKV, D)
    wv4 = wv.reshape(E, HKV, D)
    wo4 = wo.reshape(HQ, D, E)

    in_maps = []
    for c in range(NCORE):
        qh = slice(HL * c, HL * (c + 1))
        wq_c = np.ascontiguousarray(wq4[:, qh, :][:, :, perm]).reshape(E, HL * D).astype(bf)
        wk_c = np.ascontiguousarray(wk4[:, c, perm]).astype(bf)
        wv_c = np.ascontiguousarray(wv4[:, c, :]).astype(bf)
        wo_c = np.ascontiguousarray(wo4[qh].reshape(HL * D, E)).astype(bf)
        in_maps.append({
            names["xt"]: xt, names["wq"]: wq_c, names["wk"]: wk_c,
            names["wv"]: wv_c, names["wo"]: wo_c, names["cosg"]: cosg,
            names["sing"]: sing, names["maskb"]: maskb,
        })

    res = run_bass_kernel_spmd(nc, in_maps, core_ids=list(range(NCORE)))
    out = np.zeros((TOK, E), dtype=np.float32)
    for c in range(NCORE):
        out += res.results[c][names["out"]]
    return out.reshape(B, S, E)


# revision 14
# speedup vs baseline: 21328.7474x; 21328.7474x over previous
"""GQA attention layer with RoPE + KV cache (LLaMA-70B style) on 8 Trainium2 cores.

Sharding: tensor-parallel over heads. Each core owns 4 q heads + 1 kv head
(one whole GQA group) and the matching column-shards of wq/wk/wv and row-shard
of wo. Every core sees the full batch/sequence, computes its partial output
projection, and the host sums the 8 partials (the TP all-reduce) and reshapes.

Device pipeline per core (all matmuls bf16, fp32 psum accumulation):
  1. Projections: q^T/k^T/v^T = W^T x^T streamed over token blocks; RoPE
     applied on DVE with a host-side head-dim permutation that makes the
     rotate-half partner reachable by stream_shuffle (+/-16 within each
     32-partition quadrant); v transposed to natural [tok, D] layout on PE.
  2. Attention per (batch, head): s = q^T.T @ k^T in [qt,kt] layout with the
     causal staircase masked by accumulating a -30000 bias tile via an
     identity matmul; exp+row-sum fused on ScalarE (accum_out); probabilities
     normalized on DVE, transposed on PE, then av^T = v.T @ p^T accumulated
     over kt blocks.
  3. Output projection: out[tok, E] += av_h^T.T @ wo_h accumulated over the
     4 local heads; partial written to HBM in fp32.
"""
import numpy as np
import ml_dtypes

B, S, E = 2, 2048, 4096
HQ, HKV, D = 32, 8, 128
NCORE = 8
HL = HQ // NCORE          # 4 local q heads
TOK = B * S               # 4096 tokens, b-major
SCALE = 1.0 / np.sqrt(E)
TB = 512                  # token block for projections
NTB = TOK // TB           # 8
NEG = -30000.0

_CACHE = {}


def _perm_src():
    """partition p <- original head-dim index, RoPE pairs at (p, p+16) in quadrant."""
    perm = np.zeros(D, dtype=np.int64)
    for j in range(64):
        b_, s_ = j // 16, j % 16
        perm[32 * b_ + s_] = 2 * j
        perm[32 * b_ + 16 + s_] = 2 * j + 1
    return perm


def _build():
    import concourse.mybir as mybir
    import concourse.tile as tile
    from concourse import bacc
    from concourse.masks import make_identity

    bf = mybir.dt.bfloat16
    f32 = mybir.dt.float32
    Exp = mybir.ActivationFunctionType.Exp
    mult = mybir.AluOpType.mult
    add = mybir.AluOpType.add
    AX = mybir.AxisListType.X

    nc = bacc.Bacc(debug=False)
    with tile.TileContext(nc) as tc:
        with tc.tile_pool(name="dram", bufs=1, space="DRAM") as dram:
            d_xt = dram.tile([NTB, E, TB], bf, kind="ExternalInput", name="xt")
            d_wq = dram.tile([E, HL * D], bf, kind="ExternalInput", name="wq")
            d_wk = dram.tile([E, D], bf, kind="ExternalInput", name="wk")
            d_wv = dram.tile([E, D], bf, kind="ExternalInput", name="wv")
            d_wo = dram.tile([HL * D, E], bf, kind="ExternalInput", name="wo")
            d_cos = dram.tile([D, S], bf, kind="ExternalInput", name="cosg")
            d_sin = dram.tile([D, S], bf, kind="ExternalInput", name="sing")
            d_msk = dram.tile([128, 128], bf, kind="ExternalInput", name="maskb")
            d_out = dram.tile([TOK, E], mybir.dt.float16, kind="ExternalOutput", name="out")

            from contextlib import ExitStack
            _stack = ExitStack()
            const = _stack.enter_context(tc.tile_pool(name="const", bufs=1))
            wpool = _stack.enter_context(tc.tile_pool(name="wpool", bufs=1))
            qkv = _stack.enter_context(tc.tile_pool(name="qkv", bufs=1))

            ident = const.tile([128, 128], bf, tag="ident")
            make_identity(nc, ident[:])
            maskb = const.tile([128, 128], bf, tag="maskb")
            nc.sync.dma_start(out=maskb[:], in_=d_msk[:])
            wq_sb = wpool.tile([128, E // 128, HL * D], bf, tag="wq")
            wk_sb = wpool.tile([128, E // 128, D], bf, tag="wk")
            wv_sb = wpool.tile([128, E // 128, D], bf, tag="wv")
            nc.sync.dma_start(out=wq_sb[:], in_=d_wq[:].rearrange("(o p) m -> p o m", p=128))
            nc.sync.dma_start(out=wk_sb[:], in_=d_wk[:].rearrange("(o p) m -> p o m", p=128))
            nc.sync.dma_start(out=wv_sb[:], in_=d_wv[:].rearrange("(o p) m -> p o m", p=128))

            # phase-1 outputs (resident through the whole kernel)
            qT = [qkv.tile([D, TOK], bf, tag=f"qT{h}", name=f"qT{h}") for h in range(HL)]
            kT = qkv.tile([D, TOK], bf, tag="kT")
            vn = qkv.tile([128, TOK], bf, tag="vn")  # [tok%128, 128*tile + d]

            NE = E // 128  # 32 contraction chunks
            shuf = [(i + 16) % 32 for i in range(32)]

            # ---------------- Phase 1: projections + RoPE ----------------
            with (
                tc.tile_pool(name="xt", bufs=NE + 4) as xtp,
                tc.tile_pool(name="trig", bufs=1) as trig,
                tc.tile_pool(name="prj", bufs=3) as prj,
                tc.tile_pool(name="pps", bufs=1, space="PSUM") as pps,
                tc.tile_pool(name="vtp", bufs=1, space="PSUM") as vtp,
            ):
                cosg = trig.tile([D, S], bf, tag="cosg")
                sing = trig.tile([D, S], bf, tag="sing")
                nc.sync.dma_start(out=cosg[:], in_=d_cos[:])
                nc.sync.dma_start(out=sing[:], in_=d_sin[:])
                for t in range(NTB):
                    xts = []
                    for j in range(NE):
                        xt_t = xtp.tile([128, TB], bf, tag="xt")
                        nc.sync.dma_start(out=xt_t[:], in_=d_xt[t, 128 * j:128 * (j + 1), :])
                        xts.append(xt_t)
                    outs = []  # (psum, kind, h)
                    for h in range(HL):
                        ps = pps.tile([128, TB], f32, tag=f"q{h}")
                        for j in range(NE):
                            nc.tensor.matmul(ps[:], wq_sb[:, j, 128 * h:128 * (h + 1)],
                                             xts[j][:], start=(j == 0), stop=(j == NE - 1))
                        outs.append((ps, "q", h))
                    psk = pps.tile([128, TB], f32, tag="k")
                    for j in range(NE):
                        nc.tensor.matmul(psk[:], wk_sb[:, j, :], xts[j][:],
                                         start=(j == 0), stop=(j == NE - 1))
                    outs.append((psk, "k", 0))
                    psv = pps.tile([128, TB], f32, tag="v")
                    for j in range(NE):
                        nc.tensor.matmul(psv[:], wv_sb[:, j, :], xts[j][:],
                                         start=(j == 0), stop=(j == NE - 1))

                    sl = slice(TB * t, TB * (t + 1))
                    psl = slice((TB * t) % S, (TB * t) % S + TB)
                    for ps, kind, h in outs:
                        dst = qT[h] if kind == "q" else kT
                        praw = prj.tile([128, TB], f32, tag="praw")
                        nc.scalar.copy(out=praw[:], in_=ps[:])
                        u = prj.tile([128, TB], bf, tag="u")
                        nc.vector.tensor_tensor(out=u[:], in0=praw[:], in1=cosg[:, psl], op=mult)
                        sh = prj.tile([128, TB], f32, tag="sh")
                        nc.vector.stream_shuffle(out=sh[:], in_=praw[:], mask=shuf)
                        w_ = prj.tile([128, TB], bf, tag="w")
                        nc.vector.tensor_tensor(out=w_[:], in0=sh[:], in1=sing[:, psl], op=mult)
                        nc.vector.tensor_tensor(out=dst[:, sl], in0=u[:], in1=w_[:], op=add)
                    # v: evacuate v^T then transpose to natural [tok, D]
                    vte = prj.tile([128, TB], bf, tag="vte")
                    nc.scalar.copy(out=vte[:], in_=psv[:])
                    for q in range(TB // 128):
                        vt_ps = vtp.tile([128, 128], bf, tag="vt")
                        nc.tensor.transpose(vt_ps[:], vte[:, 128 * q:128 * (q + 1)], ident[:])
                        gt = t * (TB // 128) + q  # global token tile
                        nc.vector.tensor_copy(out=vn[:, 128 * gt:128 * (gt + 1)], in_=vt_ps[:])

            # ---------------- Phase 2 + 3 per batch ----------------
            wo_pool = _stack.enter_context(tc.tile_pool(name="wo_pool", bufs=1))
            wo_sb = wo_pool.tile([128, HL, E], bf, tag="wo")
            nc.sync.dma_start(out=wo_sb[:], in_=d_wo[:].rearrange("(h p) e -> p h e", p=128))
            att = _stack.enter_context(tc.tile_pool(name="att", bufs=2))
            sps = _stack.enter_context(tc.tile_pool(name="sps", bufs=2, space="PSUM"))
            tps = _stack.enter_context(tc.tile_pool(name="tps", bufs=2, space="PSUM"))
            aps = _stack.enter_context(tc.tile_pool(name="aps", bufs=2, space="PSUM"))
            wop = _stack.enter_context(tc.tile_pool(name="wop", bufs=3))
            ops = _stack.enter_context(tc.tile_pool(name="ops", bufs=2, space="PSUM"))
            for b_ in range(B):
                boff = b_ * S
                avT = [qkv.tile([D, S], bf, tag=f"avT{h}_{b_}", name=f"avT{h}_{b_}") for h in range(HL)]
                if True:
                    for h in range(HL):
                        for M in range(4):  # 512-token q supertile
                            pch = []
                            for i in range(4):  # 128-token q tile
                                m = 4 * M + i
                                q0 = 512 * M + 128 * i
                                p_sb = att.tile([128, 2048], bf, tag=f"p{i}")
                                dacc = att.tile([128, 4], f32, tag=f"dacc{i}")
                                for c in range(M + 1):
                                    width = 512 if c < M else 128 * (i + 1)
                                    s_ps = sps.tile([128, 512], f32, tag="s")
                                    nc.tensor.matmul(
                                        s_ps[:, :width],
                                        qT[h][:, boff + q0: boff + q0 + 128],
                                        kT[:, boff + 512 * c: boff + 512 * c + width],
                                        start=True, stop=(c < M))
                                    if c == M:
                                        nc.tensor.matmul(
                                            s_ps[:, 128 * i:128 * (i + 1)],
                                            ident[:], maskb[:], start=False, stop=True)
                                    nc.scalar.activation(
                                        out=p_sb[:, 512 * c: 512 * c + width],
                                        in_=s_ps[:, :width], func=Exp, scale=float(SCALE),
                                        accum_out=dacc[:, c:c + 1])
                                if i < 3:
                                    nc.vector.memset(p_sb[:, 512 * M + 128 * (i + 1): 512 * (M + 1)], 0.0)
                                dsum = att.tile([128, 1], f32, tag=f"ds{i}")
                                if M > 0:
                                    nc.vector.tensor_reduce(out=dsum[:], in_=dacc[:, :M + 1], axis=AX, op=add)
                                else:
                                    nc.vector.tensor_copy(out=dsum[:], in_=dacc[:, 0:1])
                                rcp = att.tile([128, 1], f32, tag=f"rcp{i}")
                                nc.vector.reciprocal(rcp[:], dsum[:])
                                nc.vector.tensor_scalar(
                                    out=p_sb[:, :512 * (M + 1)], in0=p_sb[:, :512 * (M + 1)],
                                    scalar1=rcp[:, 0:1], scalar2=None, op0=mult)
                                pch.append(p_sb)
                            av_ps = aps.tile([128, 512], f32, tag="av")
                            nk = 4 * (M + 1)
                            for jkt in range(nk):
                                pt_ps = tps.tile([128, 512], bf, tag="pt")
                                for i in range(4):
                                    nc.tensor.transpose(
                                        pt_ps[:, 128 * i:128 * (i + 1)],
                                        pch[i][:, 128 * jkt:128 * (jkt + 1)], ident[:])
                                pt_sb = att.tile([128, 512], bf, tag="pt_sb")
                                nc.vector.tensor_copy(out=pt_sb[:], in_=pt_ps[:])
                                gkt = (boff // 128) + jkt
                                nc.tensor.matmul(av_ps[:], vn[:, 128 * gkt:128 * (gkt + 1)],
                                                 pt_sb[:], start=(jkt == 0), stop=(jkt == nk - 1))
                            nc.vector.tensor_copy(out=avT[h][:, 512 * M:512 * (M + 1)], in_=av_ps[:])

                # ---- Phase 3: out[tok, E] = sum_h avT_h.T @ wo_h ----
                if True:
                    for tt in range(S // 128):
                        for e in range(E // 512):
                            o_ps = ops.tile([128, 512], f32, tag="o")
                            for h in range(HL):
                                nc.tensor.matmul(
                                    o_ps[:], avT[h][:, 128 * tt:128 * (tt + 1)],
                                    wo_sb[:, h, 512 * e:512 * (e + 1)],
                                    start=(h == 0), stop=(h == HL - 1))
                            o_sb = wop.tile([128, 512], mybir.dt.float16, tag="o_sb")
                            nc.vector.tensor_copy(out=o_sb[:], in_=o_ps[:])
                            nc.sync.dma_start(
                                out=d_out[boff + 128 * tt: boff + 128 * (tt + 1),
                                          512 * e:512 * (e + 1)],
                                in_=o_sb[:])
            _stack.close()
    names = {k: t.tensor.name for k, t in dict(
        xt=d_xt, wq=d_wq, wk=d_wk, wv=d_wv, wo=d_wo,
        cosg=d_cos, sing=d_sin, maskb=d_msk, out=d_out).items()}
    nc.compile()
    return nc, names


def kernel(x, input_pos, freqs_cis, wq, wk, wv, wo, k_cache=None, v_cache=None, **_):
    from concourse.bass_utils import run_bass_kernel_spmd

    x = np.asarray(x, dtype=np.float32)
    freqs_cis = np.asarray(freqs_cis, dtype=np.float32)
    wq = np.asarray(wq, dtype=np.float32)
    wk = np.asarray(wk, dtype=np.float32)
    wv = np.asarray(wv, dtype=np.float32)
    wo = np.asarray(wo, dtype=np.float32)
    bf = ml_dtypes.bfloat16

    if "nc" not in _CACHE:
        _CACHE["nc"] = _build()
    nc, names = _CACHE["nc"]

    perm = _perm_src()
    # xt blocks: [NTB, E, TB] bf16
    x2 = x.reshape(TOK, E)
    xt = np.ascontiguousarray(
        x2.T.reshape(E, NTB, TB).transpose(1, 0, 2)).astype(bf)
    # cos/sin grids in permuted-partition layout
    j_of_p = 16 * (np.arange(D) // 32) + (np.arange(D) % 16)
    sign = np.where((np.arange(D) % 32) < 16, -1.0, 1.0).astype(np.float32)
    cosg = freqs_cis[:, :, 0].T[j_of_p, :].astype(bf)                  # [D, S]
    sing = (freqs_cis[:, :, 1].T[j_of_p, :] * sign[:, None]).astype(bf)
    maskb = np.where(np.arange(128)[None, :] <= np.arange(128)[:, None],
                     0.0, NEG).astype(bf)

    wq4 = wq.reshape(E, HQ, D)
    wk4 = wk.reshape(E, HKV, D)
    wv4 = wv.reshape(E, HKV, D)
    wo4 = wo.reshape(HQ, D, E)

    in_maps = []
    for c in range(NCORE):
        qh = slice(HL * c, HL * (c + 1))
        wq_c = np.ascontiguousarray(wq4[:, qh, :][:, :, perm]).reshape(E, HL * D).astype(bf)
        wk_c = np.ascontiguousarray(wk4[:, c, perm]).astype(bf)
        wv_c = np.ascontiguousarray(wv4[:, c, :]).astype(bf)
        wo_c = np.ascontiguousarray(wo4[qh].reshape(HL * D, E)).astype(bf)
        in_maps.append({
            names["xt"]: xt, names["wq"]: wq_c, names["wk"]: wk_c,
            names["wv"]: wv_c, names["wo"]: wo_c, names["cosg"]: cosg,
            names["sing"]: sing, names["maskb"]: maskb,
        })

    res = run_bass_kernel_spmd(nc, in_maps, core_ids=list(range(NCORE)))
    out = np.zeros((TOK, E), dtype=np.float32)
    for c in range(NCORE):
        out += res.results[c][names["out"]].astype(np.float32)
    return out.reshape(B, S, E)
